# revision 40
# baseline (speedup 1.0000x reference)
"""Trainium2 Bass kernel for nn_BigramBaseline: causal mean pooling over
embedding-gathered rows.  (v2 snapshot: measured 112158 ns, rel err 2.29e-3)

  logits[b*T + t, :] = mean_{s<=t} emb[idx[b, s], :]

Strategy (data-parallel over batch, one batch row per core):
  - emb is cast to bf16 on host; the device gathers bf16 rows and writes
    bf16 outputs (upcast to f32 on host). Rounding error ~0.3% rel, well
    under the 2e-2 gate, and it halves HBM traffic both ways: 64 MiB ->
    32 MiB per core, which is what matters in this memory-bound regime.
  - per 128-token block: indirect-DMA gather of 128 emb rows -> SBUF
    tile [128, V] (partition = token within block)
  - in-block causal prefix sum via PE matmul with a lower-triangular
    ones matrix (lhsT = upper-triangular incl. diag)
  - cross-block carry kept resident in PSUM: after emitting the block's
    prefix sums, a second matmul with the strict complement mask adds
    the rest of the block's column-sums, turning the PSUM bank into
    carry_{k+1} broadcast over all 128 partitions
  - scale by 1/(t+1) during the PSUM->SBUF copy (per-partition scale
    operand); chunks 0-3 go through the scalar engine, 4-7 through the
    vector engine, splitting the copy load
  - tril/strict matmuls are batched per block (all 8 tril, then all 8
    strict) so the PE swaps weights twice per block instead of 16 times
"""

import numpy as np
import ml_dtypes

B, T, V = 8, 2048, 4096
P = 128
CHUNK = 512
N_CORES = 8

# Output quantization: int8 codes with a host-computed per-token scale
# (the host knows idx, so it can compute the exact per-token sigma of the
# prefix mean from row multiplicities). Headroom multiple covers the max of
# 4096 gaussian samples (~3.7 sigma); rel quant err = 4.6/(127*sqrt(12)) ~ 1%.
INT8_OUT = True
Q_SIGMA = 4.6


def build_bass(t=T, v=V, int8_out=INT8_OUT):
    import concourse.bacc as bacc
    import concourse.bass as bass
    import concourse.tile as tile
    from concourse import mybir

    nblk = t // P
    chunk = min(CHUNK, v)
    nchunk = v // chunk

    mm_dt = mybir.dt.bfloat16
    out_dt = mybir.dt.int8 if int8_out else mm_dt

    # 2 SWDGE queues spread the descriptor-ring load; in-flight gather count
    # is bounded by the x pool depth (dma_gather does not serialize at issue).
    nc = bacc.Bacc(trn_type="TRN2", num_swdge_queues=2)
    emb = nc.declare_dram_parameter("emb", [v, v], mm_dt, isOutput=False)
    # dma_gather index layout: flat token i of block k lives at
    # [i % 16, k*8 + i // 16], replicated across partition groups of 16.
    idx = nc.declare_dram_parameter(
        "idx", [P, nblk * (P // 16)], mybir.dt.int16, isOutput=False
    )
    invd = nc.declare_dram_parameter("invd", [P, nblk], mybir.dt.float32, isOutput=False)
    masks = nc.declare_dram_parameter("masks", [P, 2 * P], mm_dt, isOutput=False)
    out = nc.declare_dram_parameter("out", [t, v], out_dt, isOutput=True)

    with tile.TileContext(nc) as tc:
        with (
            tc.tile_pool(name="const", bufs=1) as cpool,
            tc.tile_pool(name="x", bufs=4) as xpool,
            tc.tile_pool(name="o", bufs=4) as opool,
            tc.tile_pool(name="acc", bufs=1, space="PSUM") as ppool,
        ):
            idx_sb = cpool.tile([P, nblk * (P // 16)], mybir.dt.int16)
            nc.sync.dma_start(out=idx_sb[:], in_=idx[:])
            invd_sb = cpool.tile([P, nblk], mybir.dt.float32)
            nc.sync.dma_start(out=invd_sb[:], in_=invd[:])
            masks_sb = cpool.tile([P, 2 * P], mm_dt)
            nc.sync.dma_start(out=masks_sb[:], in_=masks[:])
            trilT_sb = masks_sb[:, 0:P]
            strictT_sb = masks_sb[:, P : 2 * P]

            acc = [
                ppool.tile([P, chunk], mybir.dt.float32, name=f"acc{c}", tag=f"acc{c}")
                for c in range(nchunk)
            ]

            for w in range(16):
                nc.tensor.matmul(
                    out=acc[0][:, 0:256],
                    lhsT=trilT_sb,
                    rhs=masks_sb[:, 0:256],
                    start=True,
                    stop=True,
                    skip_group_check=True,
                )
            scratch = cpool.tile([P, 1], mybir.dt.float32)
            nc.scalar.activation(
                out=scratch[:],
                in_=invd_sb[:, 0:1],
                func=mybir.ActivationFunctionType.Copy,
            )
            scratch2 = cpool.tile([P, 1], mybir.dt.float32)
            nc.vector.tensor_scalar_mul(scratch2[:], invd_sb[:, 0:1], invd_sb[:, 0:1])

            half = v // 2
            hchunk = nchunk // 2
            gq = 0
            for k in range(nblk):
                x = xpool.tile([P, 1, v], mm_dt)
                # Two half-row gathers per block; in-flight transfers are
                # bounded by the x pool depth (4 blocks ~ 4 MiB), small
                # enough that the round-robin convoy stays short while the
                # issue chain has no gen-after-completion bubbles.
                for h in range(2):
                    csl = slice(h * half, (h + 1) * half)
                    nc.gpsimd.dma_gather(
                        x[:, :, csl],
                        emb[:, csl],
                        idx_sb[:, k * (P // 16) : (k + 1) * (P // 16)],
                        P,
                        P,
                        half,
                        elem_step=v,
                        queue_num=gq % 2,
                    )
                    gq += 1
                o = opool.tile([P, v], out_dt)
                for c in range(nchunk):
                    nc.tensor.matmul(
                        out=acc[c][:],
                        lhsT=trilT_sb,
                        rhs=x[:, 0, bass.ts(c, chunk)],
                        start=(k == 0),
                        stop=True,
                        skip_group_check=True,
                    )
                for c in range(nchunk):
                    sl = bass.ts(c, chunk)
                    if c < hchunk:
                        nc.scalar.activation(
                            out=o[:, sl],
                            in_=acc[c][:],
                            func=mybir.ActivationFunctionType.Copy,
                            scale=invd_sb[:, k : k + 1],
                        )
                    else:
                        nc.vector.tensor_scalar_mul(
                            o[:, sl], acc[c][:], invd_sb[:, k : k + 1]
                        )
                if k < nblk - 1:
                    for c in range(nchunk):
                        nc.tensor.matmul(
                            out=acc[c][:],
                            lhsT=strictT_sb,
                            rhs=x[:, 0, bass.ts(c, chunk)],
                            start=False,
                            stop=True,
                            skip_group_check=True,
                        )
                for h in range(2):
                    csl = slice(h * half, (h + 1) * half)
                    nc.sync.dma_start(
                        out=out[bass.ts(k, P), csl], in_=o[:, csl]
                    )
                nc.scalar.activation(
                    out=o[:, 0:1],
                    in_=invd_sb[:, 0:1],
                    func=mybir.ActivationFunctionType.Copy,
                )
                nc.vector.tensor_scalar_mul(
                    o[:, half : half + 1], invd_sb[:, 0:1], invd_sb[:, 0:1]
                )
    nc.finalize()
    return nc


def host_inputs(idx_row, emb_bf16, t=T, v=V, int8_out=INT8_OUT):
    """Returns (in_map, dequant_scales or None)."""
    nblk = t // P
    idx_row = np.asarray(idx_row, dtype=np.int64)
    # dma_gather wrapped layout: token i of block k -> [i % 16, k*8 + i // 16],
    # replicated across the 8 partition groups of 16.
    idx16 = (
        idx_row.astype(np.int16)
        .reshape(nblk, P // 16, 16)        # [k, i//16, i%16]
        .transpose(2, 0, 1)                # [i%16, k, i//16]
        .reshape(16, nblk * (P // 16))
    )
    idx16 = np.ascontiguousarray(np.tile(idx16, (P // 16, 1)))
    inv_t = 1.0 / np.arange(1, t + 1, dtype=np.float64)
    if int8_out:
        # Exact per-token sigma of the prefix mean: sqrt(sum of squared
        # multiplicities of the gathered rows over the causal prefix)/(t+1).
        counts = np.zeros(v, dtype=np.int64)
        sumsq = np.empty(t, dtype=np.float64)
        run = 0
        for s, r in enumerate(idx_row):
            run += 2 * counts[r] + 1
            counts[r] += 1
            sumsq[s] = run
        s_t = Q_SIGMA * np.sqrt(sumsq) * inv_t / 127.0
        q = inv_t / s_t          # device: int8 code = PSUM * q
        deq = s_t.astype(np.float32)
    else:
        q = inv_t
        deq = None
    invd = np.ascontiguousarray(q.astype(np.float32).reshape(nblk, P).T)
    masks = np.concatenate(
        [
            np.triu(np.ones((P, P), dtype=ml_dtypes.bfloat16)),
            np.tril(np.ones((P, P), dtype=ml_dtypes.bfloat16), -1),
        ],
        axis=1,
    )
    return {
        "emb": emb_bf16,
        "idx": idx16,
        "invd": invd,
        "masks": np.ascontiguousarray(masks),
    }, deq


_nc_cache = {}


def kernel(idx, emb, _trace=False):
    from concourse.bass_utils import run_bass_kernel_spmd

    key = "nc"
    if key not in _nc_cache:
        _nc_cache[key] = build_bass()
    nc = _nc_cache[key]

    idx = np.asarray(idx)
    emb_bf16 = np.ascontiguousarray(np.asarray(emb).astype(ml_dtypes.bfloat16))
    in_maps = []
    deqs = []
    for b in range(N_CORES):
        m, deq = host_inputs(idx[b], emb_bf16)
        in_maps.append(m)
        deqs.append(deq)
    res = run_bass_kernel_spmd(nc, in_maps, list(range(N_CORES)), trace=_trace)
    kernel.last_results = res
    outs = []
    for b, r in enumerate(res.results):
        o = np.asarray(r["out"])
        if INT8_OUT:
            o = o.astype(np.float32) * deqs[b][:, None]
        else:
            o = o.astype(np.float32)
        outs.append(o)
    return np.concatenate(outs, axis=0)


# revision 42
# speedup vs baseline: 1.1885x; 1.1885x over previous
"""Trainium2 Bass kernel for nn_BigramBaseline: causal mean pooling over
embedding-gathered rows.  (v2 snapshot: measured 112158 ns, rel err 2.29e-3)

  logits[b*T + t, :] = mean_{s<=t} emb[idx[b, s], :]

Strategy (data-parallel over batch, one batch row per core):
  - emb is cast to bf16 on host; the device gathers bf16 rows and writes
    bf16 outputs (upcast to f32 on host). Rounding error ~0.3% rel, well
    under the 2e-2 gate, and it halves HBM traffic both ways: 64 MiB ->
    32 MiB per core, which is what matters in this memory-bound regime.
  - per 128-token block: indirect-DMA gather of 128 emb rows -> SBUF
    tile [128, V] (partition = token within block)
  - in-block causal prefix sum via PE matmul with a lower-triangular
    ones matrix (lhsT = upper-triangular incl. diag)
  - cross-block carry kept resident in PSUM: after emitting the block's
    prefix sums, a second matmul with the strict complement mask adds
    the rest of the block's column-sums, turning the PSUM bank into
    carry_{k+1} broadcast over all 128 partitions
  - scale by 1/(t+1) during the PSUM->SBUF copy (per-partition scale
    operand); chunks 0-3 go through the scalar engine, 4-7 through the
    vector engine, splitting the copy load
  - tril/strict matmuls are batched per block (all 8 tril, then all 8
    strict) so the PE swaps weights twice per block instead of 16 times
"""

import numpy as np
import ml_dtypes

B, T, V = 8, 2048, 4096
P = 128
CHUNK = 512
N_CORES = 8

# Output quantization: int8 codes with a host-computed per-token scale
# (the host knows idx, so it can compute the exact per-token sigma of the
# prefix mean from row multiplicities). Headroom multiple covers the max of
# 4096 gaussian samples (~3.7 sigma); rel quant err = 4.6/(127*sqrt(12)) ~ 1%.
INT8_OUT = True
Q_SIGMA = 4.6


def build_bass(t=T, v=V, int8_out=INT8_OUT):
    import concourse.bacc as bacc
    import concourse.bass as bass
    import concourse.tile as tile
    from concourse import mybir

    nblk = t // P
    chunk = min(CHUNK, v)
    nchunk = v // chunk

    mm_dt = mybir.dt.bfloat16
    out_dt = mybir.dt.int8 if int8_out else mm_dt

    # Double the SWDGE descriptor ring: the default 16KB holds only two
    # 128-descriptor indirect DMAs, so the next block's descriptor
    # generation had to wait for the previous block's transfer to complete
    # (~2.6us of exposed gen+sem per block). 32KB lets the next pair
    # generate while the current pair is still in flight, without allowing
    # a deep multi-DMA convoy.
    nc = bacc.Bacc(trn_type="TRN2", dynamic_dma_scratch_size=32768)
    emb = nc.declare_dram_parameter("emb", [v, v], mm_dt, isOutput=False)
    idx = nc.declare_dram_parameter("idx", [P, nblk], mybir.dt.int32, isOutput=False)
    invd = nc.declare_dram_parameter("invd", [P, nblk], mybir.dt.float32, isOutput=False)
    masks = nc.declare_dram_parameter("masks", [P, 2 * P], mm_dt, isOutput=False)
    out = nc.declare_dram_parameter("out", [t, v], out_dt, isOutput=True)

    with tile.TileContext(nc) as tc:
        with (
            tc.tile_pool(name="const", bufs=1) as cpool,
            tc.tile_pool(name="x", bufs=4) as xpool,
            tc.tile_pool(name="o", bufs=4) as opool,
            tc.tile_pool(name="acc", bufs=1, space="PSUM") as ppool,
        ):
            idx_sb = cpool.tile([P, nblk], mybir.dt.int32)
            nc.sync.dma_start(out=idx_sb[:], in_=idx[:])
            invd_sb = cpool.tile([P, nblk], mybir.dt.float32)
            nc.sync.dma_start(out=invd_sb[:], in_=invd[:])
            masks_sb = cpool.tile([P, 2 * P], mm_dt)
            nc.sync.dma_start(out=masks_sb[:], in_=masks[:])
            trilT_sb = masks_sb[:, 0:P]
            strictT_sb = masks_sb[:, P : 2 * P]

            acc = [
                ppool.tile([P, chunk], mybir.dt.float32, name=f"acc{c}", tag=f"acc{c}")
                for c in range(nchunk)
            ]

            for w in range(16):
                nc.tensor.matmul(
                    out=acc[0][:, 0:256],
                    lhsT=trilT_sb,
                    rhs=masks_sb[:, 0:256],
                    start=True,
                    stop=True,
                    skip_group_check=True,
                )
            scratch = cpool.tile([P, 1], mybir.dt.float32)
            nc.scalar.activation(
                out=scratch[:],
                in_=invd_sb[:, 0:1],
                func=mybir.ActivationFunctionType.Copy,
            )
            scratch2 = cpool.tile([P, 1], mybir.dt.float32)
            nc.vector.tensor_scalar_mul(scratch2[:], invd_sb[:, 0:1], invd_sb[:, 0:1])

            half = v // 2
            hchunk = nchunk // 2
            for k in range(nblk):
                x = xpool.tile([P, v], mm_dt)
                for h in range(2):
                    nc.gpsimd.indirect_dma_start(
                        out=x[:, h * half : (h + 1) * half],
                        out_offset=None,
                        in_=emb[:],
                        in_offset=bass.IndirectOffsetOnAxis(
                            ap=idx_sb[:, k : k + 1], axis=0
                        ),
                        element_offset=h * half,
                    )
                o = opool.tile([P, v], out_dt)
                for c in range(nchunk):
                    nc.tensor.matmul(
                        out=acc[c][:],
                        lhsT=trilT_sb,
                        rhs=x[:, bass.ts(c, chunk)],
                        start=(k == 0),
                        stop=True,
                        skip_group_check=True,
                    )
                for c in range(nchunk):
                    sl = bass.ts(c, chunk)
                    if c < hchunk:
                        nc.scalar.activation(
                            out=o[:, sl],
                            in_=acc[c][:],
                            func=mybir.ActivationFunctionType.Copy,
                            scale=invd_sb[:, k : k + 1],
                        )
                    else:
                        nc.vector.tensor_scalar_mul(
                            o[:, sl], acc[c][:], invd_sb[:, k : k + 1]
                        )
                if k < nblk - 1:
                    for c in range(nchunk):
                        nc.tensor.matmul(
                            out=acc[c][:],
                            lhsT=strictT_sb,
                            rhs=x[:, bass.ts(c, chunk)],
                            start=False,
                            stop=True,
                            skip_group_check=True,
                        )
                for h in range(2):
                    csl = slice(h * half, (h + 1) * half)
                    nc.sync.dma_start(
                        out=out[bass.ts(k, P), csl], in_=o[:, csl]
                    )
                nc.scalar.activation(
                    out=o[:, 0:1],
                    in_=invd_sb[:, 0:1],
                    func=mybir.ActivationFunctionType.Copy,
                )
                nc.vector.tensor_scalar_mul(
                    o[:, half : half + 1], invd_sb[:, 0:1], invd_sb[:, 0:1]
                )
    nc.finalize()
    return nc


def host_inputs(idx_row, emb_bf16, t=T, v=V, int8_out=INT8_OUT):
    """Returns (in_map, dequant_scales or None)."""
    nblk = t // P
    idx_row = np.asarray(idx_row, dtype=np.int64)
    idx32 = np.ascontiguousarray(idx_row.astype(np.int32).reshape(nblk, P).T)
    inv_t = 1.0 / np.arange(1, t + 1, dtype=np.float64)
    if int8_out:
        # Exact per-token sigma of the prefix mean: sqrt(sum of squared
        # multiplicities of the gathered rows over the causal prefix)/(t+1).
        counts = np.zeros(v, dtype=np.int64)
        sumsq = np.empty(t, dtype=np.float64)
        run = 0
        for s, r in enumerate(idx_row):
            run += 2 * counts[r] + 1
            counts[r] += 1
            sumsq[s] = run
        s_t = Q_SIGMA * np.sqrt(sumsq) * inv_t / 127.0
        q = inv_t / s_t          # device: int8 code = PSUM * q
        deq = s_t.astype(np.float32)
    else:
        q = inv_t
        deq = None
    invd = np.ascontiguousarray(q.astype(np.float32).reshape(nblk, P).T)
    masks = np.concatenate(
        [
            np.triu(np.ones((P, P), dtype=ml_dtypes.bfloat16)),
            np.tril(np.ones((P, P), dtype=ml_dtypes.bfloat16), -1),
        ],
        axis=1,
    )
    return {
        "emb": emb_bf16,
        "idx": idx32,
        "invd": invd,
        "masks": np.ascontiguousarray(masks),
    }, deq


_nc_cache = {}


def kernel(idx, emb, _trace=False):
    from concourse.bass_utils import run_bass_kernel_spmd

    key = "nc"
    if key not in _nc_cache:
        _nc_cache[key] = build_bass()
    nc = _nc_cache[key]

    idx = np.asarray(idx)
    emb_bf16 = np.ascontiguousarray(np.asarray(emb).astype(ml_dtypes.bfloat16))
    in_maps = []
    deqs = []
    for b in range(N_CORES):
        m, deq = host_inputs(idx[b], emb_bf16)
        in_maps.append(m)
        deqs.append(deq)
    res = run_bass_kernel_spmd(nc, in_maps, list(range(N_CORES)), trace=_trace)
    kernel.last_results = res
    outs = []
    for b, r in enumerate(res.results):
        o = np.asarray(r["out"])
        if INT8_OUT:
            o = o.astype(np.float32) * deqs[b][:, None]
        else:
            o = o.astype(np.float32)
        outs.append(o)
    return np.concatenate(outs, axis=0)


# revision 43
# speedup vs baseline: 1.2058x; 1.0145x over previous
"""Trainium2 Bass kernel for nn_BigramBaseline: causal mean pooling over
embedding-gathered rows.  (v2 snapshot: measured 112158 ns, rel err 2.29e-3)

  logits[b*T + t, :] = mean_{s<=t} emb[idx[b, s], :]

Strategy (data-parallel over batch, one batch row per core):
  - emb is cast to bf16 on host; the device gathers bf16 rows and writes
    bf16 outputs (upcast to f32 on host). Rounding error ~0.3% rel, well
    under the 2e-2 gate, and it halves HBM traffic both ways: 64 MiB ->
    32 MiB per core, which is what matters in this memory-bound regime.
  - per 128-token block: indirect-DMA gather of 128 emb rows -> SBUF
    tile [128, V] (partition = token within block)
  - in-block causal prefix sum via PE matmul with a lower-triangular
    ones matrix (lhsT = upper-triangular incl. diag)
  - cross-block carry kept resident in PSUM: after emitting the block's
    prefix sums, a second matmul with the strict complement mask adds
    the rest of the block's column-sums, turning the PSUM bank into
    carry_{k+1} broadcast over all 128 partitions
  - scale by 1/(t+1) during the PSUM->SBUF copy (per-partition scale
    operand); chunks 0-3 go through the scalar engine, 4-7 through the
    vector engine, splitting the copy load
  - tril/strict matmuls are batched per block (all 8 tril, then all 8
    strict) so the PE swaps weights twice per block instead of 16 times
"""

import numpy as np
import ml_dtypes

B, T, V = 8, 2048, 4096
P = 128
CHUNK = 512
N_CORES = 8

# Output quantization: int8 codes with a host-computed per-token scale
# (the host knows idx, so it can compute the exact per-token sigma of the
# prefix mean from row multiplicities). Headroom multiple covers the max of
# 4096 gaussian samples (~3.7 sigma); rel quant err = 4.6/(127*sqrt(12)) ~ 1%.
INT8_OUT = True
Q_SIGMA = 4.6


def build_bass(t=T, v=V, int8_out=INT8_OUT):
    import concourse.bacc as bacc
    import concourse.bass as bass
    import concourse.tile as tile
    from concourse import mybir

    nblk = t // P
    chunk = min(CHUNK, v)
    nchunk = v // chunk

    mm_dt = mybir.dt.bfloat16
    out_dt = mybir.dt.int8 if int8_out else mm_dt

    # Double the SWDGE descriptor ring: the default 16KB holds only two
    # 128-descriptor indirect DMAs, so the next block's descriptor
    # generation had to wait for the previous block's transfer to complete
    # (~2.6us of exposed gen+sem per block). 32KB lets the next pair
    # generate while the current pair is still in flight, without allowing
    # a deep multi-DMA convoy.
    nc = bacc.Bacc(trn_type="TRN2", dynamic_dma_scratch_size=32768)
    emb = nc.declare_dram_parameter("emb", [v, v], mm_dt, isOutput=False)
    idx = nc.declare_dram_parameter("idx", [P, nblk], mybir.dt.int32, isOutput=False)
    invd = nc.declare_dram_parameter("invd", [P, nblk], mybir.dt.float32, isOutput=False)
    masks = nc.declare_dram_parameter("masks", [P, 2 * P], mm_dt, isOutput=False)
    out = nc.declare_dram_parameter("out", [t, v], out_dt, isOutput=True)

    with tile.TileContext(nc) as tc:
        with (
            tc.tile_pool(name="const", bufs=1) as cpool,
            tc.tile_pool(name="x", bufs=4) as xpool,
            tc.tile_pool(name="o", bufs=4) as opool,
            tc.tile_pool(name="acc", bufs=1, space="PSUM") as ppool,
        ):
            idx_sb = cpool.tile([P, nblk], mybir.dt.int32)
            nc.sync.dma_start(out=idx_sb[:], in_=idx[:])
            invd_sb = cpool.tile([P, nblk], mybir.dt.float32)
            nc.sync.dma_start(out=invd_sb[:], in_=invd[:])
            masks_sb = cpool.tile([P, 2 * P], mm_dt)
            nc.sync.dma_start(out=masks_sb[:], in_=masks[:])
            trilT_sb = masks_sb[:, 0:P]
            strictT_sb = masks_sb[:, P : 2 * P]

            acc = [
                ppool.tile([P, chunk], mybir.dt.float32, name=f"acc{c}", tag=f"acc{c}")
                for c in range(nchunk)
            ]

            for w in range(16):
                nc.tensor.matmul(
                    out=acc[0][:, 0:256],
                    lhsT=trilT_sb,
                    rhs=masks_sb[:, 0:256],
                    start=True,
                    stop=True,
                    skip_group_check=True,
                )
            scratch = cpool.tile([P, 1], mybir.dt.float32)
            nc.scalar.activation(
                out=scratch[:],
                in_=invd_sb[:, 0:1],
                func=mybir.ActivationFunctionType.Copy,
            )
            scratch2 = cpool.tile([P, 1], mybir.dt.float32)
            nc.vector.tensor_scalar_mul(scratch2[:], invd_sb[:, 0:1], invd_sb[:, 0:1])

            half = v // 2
            hchunk = nchunk // 2
            for k in range(nblk):
                x = xpool.tile([P, v], mm_dt)
                for h in range(2):
                    nc.gpsimd.indirect_dma_start(
                        out=x[:, h * half : (h + 1) * half],
                        out_offset=None,
                        in_=emb[:],
                        in_offset=bass.IndirectOffsetOnAxis(
                            ap=idx_sb[:, k : k + 1], axis=0
                        ),
                        element_offset=h * half,
                    )
                o = opool.tile([P, v], out_dt)
                for c in range(nchunk):
                    nc.tensor.matmul(
                        out=acc[c][:],
                        lhsT=trilT_sb,
                        rhs=x[:, bass.ts(c, chunk)],
                        start=(k == 0),
                        stop=True,
                        skip_group_check=True,
                    )
                for c in range(nchunk):
                    sl = bass.ts(c, chunk)
                    if c < hchunk:
                        nc.scalar.activation(
                            out=o[:, sl],
                            in_=acc[c][:],
                            func=mybir.ActivationFunctionType.Copy,
                            scale=invd_sb[:, k : k + 1],
                        )
                    else:
                        nc.vector.tensor_scalar_mul(
                            o[:, sl], acc[c][:], invd_sb[:, k : k + 1]
                        )
                if k < nblk - 1:
                    # Alternate ACT-copied (0-3) and DVE-copied (4-7) chunks:
                    # strict(c) waits for copy(c), and the two copy engines
                    # drain their four chunks in parallel, so this order makes
                    # each strict's copy already complete when the in-order PE
                    # queue reaches it (the 0..7 order stalls on the scalar
                    # engine's third and fourth copies).
                    for c in (0, 4, 1, 5, 2, 6, 3, 7):
                        nc.tensor.matmul(
                            out=acc[c][:],
                            lhsT=strictT_sb,
                            rhs=x[:, bass.ts(c, chunk)],
                            start=False,
                            stop=True,
                            skip_group_check=True,
                        )
                for h in range(2):
                    csl = slice(h * half, (h + 1) * half)
                    nc.sync.dma_start(
                        out=out[bass.ts(k, P), csl], in_=o[:, csl]
                    )
                nc.scalar.activation(
                    out=o[:, 0:1],
                    in_=invd_sb[:, 0:1],
                    func=mybir.ActivationFunctionType.Copy,
                )
                nc.vector.tensor_scalar_mul(
                    o[:, half : half + 1], invd_sb[:, 0:1], invd_sb[:, 0:1]
                )
    nc.finalize()
    return nc


def host_inputs(idx_row, emb_bf16, t=T, v=V, int8_out=INT8_OUT):
    """Returns (in_map, dequant_scales or None)."""
    nblk = t // P
    idx_row = np.asarray(idx_row, dtype=np.int64)
    idx32 = np.ascontiguousarray(idx_row.astype(np.int32).reshape(nblk, P).T)
    inv_t = 1.0 / np.arange(1, t + 1, dtype=np.float64)
    if int8_out:
        # Exact per-token sigma of the prefix mean: sqrt(sum of squared
        # multiplicities of the gathered rows over the causal prefix)/(t+1).
        counts = np.zeros(v, dtype=np.int64)
        sumsq = np.empty(t, dtype=np.float64)
        run = 0
        for s, r in enumerate(idx_row):
            run += 2 * counts[r] + 1
            counts[r] += 1
            sumsq[s] = run
        s_t = Q_SIGMA * np.sqrt(sumsq) * inv_t / 127.0
        q = inv_t / s_t          # device: int8 code = PSUM * q
        deq = s_t.astype(np.float32)
    else:
        q = inv_t
        deq = None
    invd = np.ascontiguousarray(q.astype(np.float32).reshape(nblk, P).T)
    masks = np.concatenate(
        [
            np.triu(np.ones((P, P), dtype=ml_dtypes.bfloat16)),
            np.tril(np.ones((P, P), dtype=ml_dtypes.bfloat16), -1),
        ],
        axis=1,
    )
    return {
        "emb": emb_bf16,
        "idx": idx32,
        "invd": invd,
        "masks": np.ascontiguousarray(masks),
    }, deq


_nc_cache = {}


def kernel(idx, emb, _trace=False):
    from concourse.bass_utils import run_bass_kernel_spmd

    key = "nc"
    if key not in _nc_cache:
        _nc_cache[key] = build_bass()
    nc = _nc_cache[key]

    idx = np.asarray(idx)
    emb_bf16 = np.ascontiguousarray(np.asarray(emb).astype(ml_dtypes.bfloat16))
    in_maps = []
    deqs = []
    for b in range(N_CORES):
        m, deq = host_inputs(idx[b], emb_bf16)
        in_maps.append(m)
        deqs.append(deq)
    res = run_bass_kernel_spmd(nc, in_maps, list(range(N_CORES)), trace=_trace)
    kernel.last_results = res
    outs = []
    for b, r in enumerate(res.results):
        o = np.asarray(r["out"])
        if INT8_OUT:
            o = o.astype(np.float32) * deqs[b][:, None]
        else:
            o = o.astype(np.float32)
        outs.append(o)
    return np.concatenate(outs, axis=0)


# revision 44
# speedup vs baseline: 1.2175x; 1.0097x over previous
"""Trainium2 Bass kernel for nn_BigramBaseline: causal mean pooling over
embedding-gathered rows.  (v2 snapshot: measured 112158 ns, rel err 2.29e-3)

  logits[b*T + t, :] = mean_{s<=t} emb[idx[b, s], :]

Strategy (data-parallel over batch, one batch row per core):
  - emb is cast to bf16 on host; the device gathers bf16 rows and writes
    bf16 outputs (upcast to f32 on host). Rounding error ~0.3% rel, well
    under the 2e-2 gate, and it halves HBM traffic both ways: 64 MiB ->
    32 MiB per core, which is what matters in this memory-bound regime.
  - per 128-token block: indirect-DMA gather of 128 emb rows -> SBUF
    tile [128, V] (partition = token within block)
  - in-block causal prefix sum via PE matmul with a lower-triangular
    ones matrix (lhsT = upper-triangular incl. diag)
  - cross-block carry kept resident in PSUM: after emitting the block's
    prefix sums, a second matmul with the strict complement mask adds
    the rest of the block's column-sums, turning the PSUM bank into
    carry_{k+1} broadcast over all 128 partitions
  - scale by 1/(t+1) during the PSUM->SBUF copy (per-partition scale
    operand); chunks 0-3 go through the scalar engine, 4-7 through the
    vector engine, splitting the copy load
  - tril/strict matmuls are batched per block (all 8 tril, then all 8
    strict) so the PE swaps weights twice per block instead of 16 times
"""

import numpy as np
import ml_dtypes

B, T, V = 8, 2048, 4096
P = 128
CHUNK = 512
N_CORES = 8


def build_bass(t=T, v=V):
    import concourse.bacc as bacc
    import concourse.bass as bass
    import concourse.tile as tile
    from concourse import mybir

    nblk = t // P
    chunk = min(CHUNK, v)
    nchunk = v // chunk

    mm_dt = mybir.dt.bfloat16

    nc = bacc.Bacc(trn_type="TRN2")
    emb = nc.declare_dram_parameter("emb", [v, v], mm_dt, isOutput=False)
    idx = nc.declare_dram_parameter("idx", [P, nblk], mybir.dt.int32, isOutput=False)
    invd = nc.declare_dram_parameter("invd", [P, nblk], mybir.dt.float32, isOutput=False)
    masks = nc.declare_dram_parameter("masks", [P, 2 * P], mm_dt, isOutput=False)
    out = nc.declare_dram_parameter("out", [t, v], mm_dt, isOutput=True)

    with tile.TileContext(nc) as tc:
        with (
            tc.tile_pool(name="const", bufs=1) as cpool,
            tc.tile_pool(name="x", bufs=4) as xpool,
            tc.tile_pool(name="o", bufs=4) as opool,
            tc.tile_pool(name="acc", bufs=1, space="PSUM") as ppool,
        ):
            idx_sb = cpool.tile([P, nblk], mybir.dt.int32)
            nc.sync.dma_start(out=idx_sb[:], in_=idx[:])
            invd_sb = cpool.tile([P, nblk], mybir.dt.float32)
            nc.sync.dma_start(out=invd_sb[:], in_=invd[:])
            masks_sb = cpool.tile([P, 2 * P], mm_dt)
            nc.sync.dma_start(out=masks_sb[:], in_=masks[:])
            trilT_sb = masks_sb[:, 0:P]
            strictT_sb = masks_sb[:, P : 2 * P]

            acc = [
                ppool.tile([P, chunk], mybir.dt.float32, name=f"acc{c}", tag=f"acc{c}")
                for c in range(nchunk)
            ]

            for w in range(16):
                nc.tensor.matmul(
                    out=acc[0][:, 0:256],
                    lhsT=trilT_sb,
                    rhs=masks_sb[:, 0:256],
                    start=True,
                    stop=True,
                    skip_group_check=True,
                )
            scratch = cpool.tile([P, 1], mybir.dt.float32)
            nc.scalar.activation(
                out=scratch[:],
                in_=invd_sb[:, 0:1],
                func=mybir.ActivationFunctionType.Copy,
            )
            scratch2 = cpool.tile([P, 1], mybir.dt.float32)
            nc.vector.tensor_scalar_mul(scratch2[:], invd_sb[:, 0:1], invd_sb[:, 0:1])

            half = v // 2
            hchunk = nchunk // 2
            for k in range(nblk):
                x = xpool.tile([P, v], mm_dt)
                for h in range(2):
                    nc.gpsimd.indirect_dma_start(
                        out=x[:, h * half : (h + 1) * half],
                        out_offset=None,
                        in_=emb[:],
                        in_offset=bass.IndirectOffsetOnAxis(
                            ap=idx_sb[:, k : k + 1], axis=0
                        ),
                        element_offset=h * half,
                    )
                o = opool.tile([P, v], mm_dt)
                for c in range(nchunk):
                    nc.tensor.matmul(
                        out=acc[c][:],
                        lhsT=trilT_sb,
                        rhs=x[:, bass.ts(c, chunk)],
                        start=(k == 0),
                        stop=True,
                        skip_group_check=True,
                    )
                for c in range(nchunk):
                    sl = bass.ts(c, chunk)
                    if c < hchunk:
                        nc.scalar.activation(
                            out=o[:, sl],
                            in_=acc[c][:],
                            func=mybir.ActivationFunctionType.Copy,
                            scale=invd_sb[:, k : k + 1],
                        )
                    else:
                        nc.vector.tensor_scalar_mul(
                            o[:, sl], acc[c][:], invd_sb[:, k : k + 1]
                        )
                if k < nblk - 1:
                    for c in range(nchunk):
                        nc.tensor.matmul(
                            out=acc[c][:],
                            lhsT=strictT_sb,
                            rhs=x[:, bass.ts(c, chunk)],
                            start=False,
                            stop=True,
                            skip_group_check=True,
                        )
                for h in range(2):
                    csl = slice(h * half, (h + 1) * half)
                    nc.sync.dma_start(
                        out=out[bass.ts(k, P), csl], in_=o[:, csl]
                    )
                nc.scalar.activation(
                    out=o[:, 0:1],
                    in_=invd_sb[:, 0:1],
                    func=mybir.ActivationFunctionType.Copy,
                )
                nc.vector.tensor_scalar_mul(
                    o[:, half : half + 1], invd_sb[:, 0:1], invd_sb[:, 0:1]
                )
    nc.finalize()
    return nc


def host_inputs(idx_row, emb_bf16, t=T, v=V):
    nblk = t // P
    idx32 = np.ascontiguousarray(
        np.asarray(idx_row, dtype=np.int32).reshape(nblk, P).T
    )
    invd = np.ascontiguousarray(
        (1.0 / np.arange(1, t + 1, dtype=np.float64))
        .astype(np.float32)
        .reshape(nblk, P)
        .T
    )
    masks = np.concatenate(
        [
            np.triu(np.ones((P, P), dtype=ml_dtypes.bfloat16)),
            np.tril(np.ones((P, P), dtype=ml_dtypes.bfloat16), -1),
        ],
        axis=1,
    )
    return {
        "emb": emb_bf16,
        "idx": idx32,
        "invd": invd,
        "masks": np.ascontiguousarray(masks),
    }


_nc_cache = {}


def kernel(idx, emb, _trace=False):
    from concourse.bass_utils import run_bass_kernel_spmd

    key = "nc"
    if key not in _nc_cache:
        _nc_cache[key] = build_bass()
    nc = _nc_cache[key]

    idx = np.asarray(idx)
    emb_bf16 = np.ascontiguousarray(np.asarray(emb).astype(ml_dtypes.bfloat16))
    in_maps = [host_inputs(idx[b], emb_bf16) for b in range(N_CORES)]
    res = run_bass_kernel_spmd(nc, in_maps, list(range(N_CORES)), trace=_trace)
    kernel.last_results = res
    out = np.concatenate(
        [np.asarray(r["out"]).astype(np.float32) for r in res.results], axis=0
    )
    return out
